# revision 8
# baseline (speedup 1.0000x reference)
"""ExLlama q4 dequant + matmul (tensor-parallel over out_features) on 8 trn2 cores.

Math (per core, N_loc = 28672/8 = 3584 columns):
  out[t,n] = sum_k x[t,k] * s[g(k),n] * (q[k,n] - (z[g(k),n]+1)) + bias[n]
           = sum_k x[t,k]*s[g,n]*q[k,n]  -  sum_g A[t,g]*(z+1)[g,n]*s[g,n] + bias[n]
  with A[t,g] = sum_{k in g} x[t,k] (host-computed, tiny).

Device pipeline per core ("Design W2" — x stationary, dequantized weights moving,
big-chunk DMA):
  - qweight host-permuted into 16 container tiles [128, N_loc] u16 (partition p
    holds k of group p//2; nibble c of word (jt,p,n) is k = (p//2)*128 +
    (jt*2+p%2)*4 + c), then packed per-partition-contiguous as
    [128, NJT*N_loc] and DMA'd in NDMA big chunks (~3.7 MB each).
  - All constants (sexp | xt | z65 | r65) packed into ONE [128, 9248] fp16
    tensor, one DMA.
  - DVE extract (4x): (u16 & (0xF<<4c)) -> u16 = q*16^c; DVE scale (2x):
    tensor_tensor mult with sexp -> w~ fp16.
  - PE: stationary = xt slice [128,32] (x*16^-c permuted), moving = w~ in 7
    chunks of N=512; PSUM [32, 3584] accumulates over all 64 passes + fixup
    matmul (lhsT=r65=[-A.T;1], rhs=z65=[(z+1)*s;bias]).
  - One ScalarE drain PSUM->SBUF fp16, one out DMA [32, 3584].
"""

import numpy as np

GROUP_SIZE = 128
IN_FEATURES = 8192
OUT_FEATURES = 28672
TOKENS = 32
N_CORES = 8
N_LOC = OUT_FEATURES // N_CORES          # 3584
NJT = IN_FEATURES // (GROUP_SIZE * 4)    # 16 container tiles
G = IN_FEATURES // GROUP_SIZE            # 64 groups
MASKS = (0x000F, 0x00F0, 0x0F00, 0xF000)
import os
MMCH = 512                               # moving cols per matmul (1 PSUM bank)
NDMA = int(os.environ.get("KM_NDMA", "4"))   # wq DMA chunks (NJT % NDMA == 0)
JPC = NJT // NDMA                        # jt tiles per chunk

# packed const layout (columns, fp16)
C_SEXP = 0
C_XT = N_LOC                             # 3584
C_Z65 = C_XT + NJT * 4 * TOKENS          # 5632
C_R65 = C_Z65 + N_LOC                    # 9216
C_W = C_R65 + TOKENS                     # 9248

_PROGRAM_CACHE = {}


# ---------------------------------------------------------------- host prep

def _k_index_map():
    """k(jt, p, c) = (p//2)*128 + (jt*2 + p%2)*4 + c  -> [NJT, 128, 4] int."""
    jt = np.arange(NJT)[:, None, None]
    p = np.arange(128)[None, :, None]
    c = np.arange(4)[None, None, :]
    return (p // 2) * GROUP_SIZE + (jt * 2 + (p % 2)) * 4 + c


def _prep_wq(qw_slice):
    """[1024, N_loc] int32 -> [128, NJT*N_loc] uint16 packed container tiles."""
    nloc = qw_slice.shape[1]
    qb = np.ascontiguousarray(qw_slice).view(np.uint8).reshape(1024, nloc, 4)
    # byte kp = 4*kk + b holds nibbles for k = 2kp (lo), 2kp+1 (hi)
    qb_kp = np.ascontiguousarray(qb.transpose(0, 2, 1)).reshape(4096, nloc)
    jt = np.arange(NJT)[:, None]
    p = np.arange(128)[None, :]
    kp0 = (p // 2) * 64 + (jt * 2 + (p % 2)) * 2      # [NJT, 128]
    b2 = np.stack([qb_kp[kp0], qb_kp[kp0 + 1]], axis=-1)  # [NJT,128,nloc,2] u8
    wq = np.ascontiguousarray(b2).view(np.uint16)[..., 0]  # [NJT,128,nloc]
    # [NDMA, 128, JPC*nloc]: each chunk fully contiguous in DRAM
    wq = wq.reshape(NDMA, JPC, 128, nloc).transpose(0, 2, 1, 3)
    return np.ascontiguousarray(wq).reshape(NDMA, 128, JPC * nloc)


def _prep_const(x, qz_slice, s_slice, b_slice):
    """Pack [sexp | xt | z65 | r65] into one [128, C_W] fp16 array."""
    nloc = s_slice.shape[1]
    cst = np.zeros((128, C_W), dtype=np.float16)
    # sexp
    cst[:, C_SEXP:C_SEXP + nloc] = np.repeat(
        s_slice.astype(np.float16), 2, axis=0)
    # xt
    kmap = _k_index_map()
    xf = x.astype(np.float32)
    for jt in range(NJT):
        for c in range(4):
            blk = xf[:, kmap[jt, :, c]].T * (2.0 ** (-4 * c))   # [128, 32]
            col = C_XT + (jt * 4 + c) * TOKENS
            cst[:, col:col + TOKENS] = blk.astype(np.float16)
    # z65 rows 0..63 = (z+1)*s, row 64 = bias
    shifts = (np.arange(8, dtype=np.uint32) * 4)[None, None, :]
    z = ((qz_slice.astype(np.uint32)[:, :, None] >> shifts) & 15)
    z = z.reshape(G, nloc).astype(np.float32)
    cst[:G, C_Z65:C_Z65 + nloc] = ((z + 1.0) * s_slice.astype(np.float32)
                                   ).astype(np.float16)
    cst[G, C_Z65:C_Z65 + nloc] = b_slice
    # r65 rows 0..63 = -A.T, row 64 = ones
    A = x.astype(np.float32).reshape(TOKENS, G, GROUP_SIZE).sum(axis=2)
    cst[:G, C_R65:C_R65 + TOKENS] = (-A.T).astype(np.float16)
    cst[G, C_R65:C_R65 + TOKENS] = 1.0
    return cst


# ---------------------------------------------------------------- device program

GP_EVERY = int(os.environ.get("KM_GP_EVERY", "0"))  # Nth scale-mult on GpSimd


def _build_program(nloc, loop_r=1, gp_every=GP_EVERY):
    import concourse.bacc as bacc
    import concourse.mybir as mybir
    import concourse.tile as tile
    from concourse.alu_op_type import AluOpType

    dt = mybir.dt
    nch = nloc // MMCH

    nc = bacc.Bacc("TRN2", target_bir_lowering=False, debug=False,
                   num_devices=N_CORES)

    wq_d = nc.dram_tensor("wq", [NDMA, 128, JPC * nloc], dt.uint16,
                          kind="ExternalInput")
    cst_d = nc.dram_tensor("cst", [128, C_W], dt.float16,
                           kind="ExternalInput")
    out_d = nc.dram_tensor("out", [TOKENS, nloc], dt.float16,
                           kind="ExternalOutput")

    with tile.TileContext(nc) as tc:
        with (
            tc.tile_pool(name="const", bufs=1) as const_pool,
            tc.tile_pool(name="wq", bufs=2) as wq_pool,
            tc.tile_pool(name="ext", bufs=4) as ext_pool,
            tc.tile_pool(name="sw", bufs=6) as sw_pool,
            tc.tile_pool(name="psum", bufs=1, space="PSUM") as psum_pool,
        ):
            def emit_body():
                cst = const_pool.tile([128, C_W], dt.float16, tag="cst")
                nc.sync.dma_start(cst[:], cst_d[:])
                sexp = cst[:, C_SEXP:C_SEXP + nloc]

                psum = psum_pool.tile([TOKENS, nch * MMCH], dt.float32,
                                      tag="acc")

                for dc in range(NDMA):
                    wq_t = wq_pool.tile([128, JPC * nloc], dt.uint16)
                    nc.sync.dma_start(wq_t[:], wq_d[dc, :, :])
                    for j in range(JPC):
                        jt = dc * JPC + j
                        for c in range(4):
                            ext = ext_pool.tile([128, nloc], dt.uint16)
                            nc.vector.tensor_scalar(
                                ext[:], wq_t[:, j * nloc:(j + 1) * nloc],
                                MASKS[c], None, AluOpType.bitwise_and)
                            sw = sw_pool.tile([128, nloc], dt.float16)
                            ip = jt * 4 + c
                            eng = (nc.gpsimd if gp_every and
                                   ip % gp_every == gp_every - 1
                                   else nc.vector)
                            eng.tensor_tensor(
                                sw[:], ext[:], sexp, AluOpType.mult)
                            xcol = C_XT + ip * TOKENS
                            for ci in range(nch):
                                nc.tensor.matmul(
                                    psum[:, ci * MMCH:(ci + 1) * MMCH],
                                    cst[:, xcol:xcol + TOKENS],
                                    sw[:, ci * MMCH:(ci + 1) * MMCH],
                                    start=(jt == 0 and c == 0),
                                    stop=False)

                for ci in range(nch):
                    nc.tensor.matmul(
                        psum[:, ci * MMCH:(ci + 1) * MMCH],
                        cst[0:G + 1, C_R65:C_R65 + TOKENS],
                        cst[0:G + 1, C_Z65 + ci * MMCH:C_Z65 + (ci + 1) * MMCH],
                        start=False,
                        stop=True)

                stg = const_pool.tile([TOKENS, nch * MMCH], dt.float16,
                                      tag="stg")
                nc.scalar.copy(stg[:], psum[:])
                nc.sync.dma_start(out_d[:], stg[:])

            if loop_r == 1:
                emit_body()
            else:
                with tc.For_i(0, loop_r, 1):
                    emit_body()

    nc.compile()
    return nc


def _get_program(nloc=N_LOC):
    if nloc not in _PROGRAM_CACHE:
        _PROGRAM_CACHE[nloc] = _build_program(nloc)
    return _PROGRAM_CACHE[nloc]


# ---------------------------------------------------------------- entry point

def make_in_maps(x, qweight, qzeros, scales, bias, nloc=N_LOC, n_cores=N_CORES):
    x = np.asarray(x)
    qweight = np.asarray(qweight)
    qzeros = np.asarray(qzeros)
    scales = np.asarray(scales)
    bias = np.asarray(bias)

    in_maps = []
    for core in range(n_cores):
        n0, n1 = core * nloc, (core + 1) * nloc
        s_slice = np.ascontiguousarray(scales[:, n0:n1]).astype(np.float16)
        qz_slice = np.ascontiguousarray(qzeros[:, n0 // 8:n1 // 8]).view(
            np.uint32)
        b_slice = np.ascontiguousarray(bias[n0:n1]).astype(np.float16)
        in_maps.append({
            "wq": _prep_wq(qweight[:, n0:n1]),
            "cst": _prep_const(x, qz_slice, s_slice, b_slice),
        })
    return in_maps


def assemble_output(results, nloc=N_LOC, n_cores=N_CORES):
    parts = [np.asarray(results[core]["out"]) for core in range(n_cores)]
    return np.ascontiguousarray(np.concatenate(parts, axis=1))


def kernel(x, qweight, qzeros, scales, bias):
    from concourse.bass_utils import run_bass_kernel_spmd

    nc = _get_program()
    in_maps = make_in_maps(x, qweight, qzeros, scales, bias)
    res = run_bass_kernel_spmd(nc, in_maps, list(range(N_CORES)))
    return assemble_output(res.results)


# revision 9
# speedup vs baseline: 1.4100x; 1.4100x over previous
"""ExLlama q4 dequant + matmul (tensor-parallel over out_features) on 8 trn2 cores.

Math (per core, N_loc = 28672/8 = 3584 columns):
  out[t,n] = sum_k x[t,k] * s[g(k),n] * (q[k,n] - (z[g(k),n]+1)) + bias[n]
           = sum_k x[t,k]*s[g,n]*q[k,n]  -  sum_g A[t,g]*(z+1)[g,n]*s[g,n] + bias[n]
  with A[t,g] = sum_{k in g} x[t,k] (host-computed, tiny).

Device pipeline per core:
  - qweight bytes are host-permuted into 16 "container tiles" [128, N_loc] uint16.
    Partition p of every tile holds k-values of group g = p//2 only, so ONE
    resident scale tile S_exp[p,n] = s[p//2, n] serves every tile.
    Container u16 at (jt, p, n) packs 4 nibbles: k = (p//2)*128 + (jt*2+p%2)*4 + c
    at bits 4c.
  - DVE extract (4x mode): (u16 & (0xF<<4c)) -> fp16 = q * 16^c exact.
  - DVE scale (2x mode): tensor_tensor mult with S_exp -> w~ = q*s*16^c fp16.
  - PE: w~ n-chunks stationary (FWL), x~ = x[t,k]*16^-c permuted tiles moving;
    PSUM [128 n, 32 t] accumulates over all 64 (jt,c) weight tiles.
  - Zero/bias fixup: one extra accumulating matmul per n-chunk with
    lhsT = [z1s; bias] (z1s dequantized on device from qzeros), rhs = [-A.T; 1].
"""

import numpy as np

GROUP_SIZE = 128
IN_FEATURES = 8192
OUT_FEATURES = 28672
TOKENS = 32
N_CORES = 8
N_LOC = OUT_FEATURES // N_CORES          # 3584
NJT = IN_FEATURES // (GROUP_SIZE * 4)    # 16 container tiles
G = IN_FEATURES // GROUP_SIZE            # 64 groups
MASKS = (0x000F, 0x00F0, 0x0F00, 0xF000)

_PROGRAM_CACHE = {}


# ---------------------------------------------------------------- host prep

def _k_index_map():
    """k(jt, p, c) = (p//2)*128 + (jt*2 + p%2)*4 + c  -> [NJT, 128, 4] int."""
    jt = np.arange(NJT)[:, None, None]
    p = np.arange(128)[None, :, None]
    c = np.arange(4)[None, None, :]
    return (p // 2) * GROUP_SIZE + (jt * 2 + (p % 2)) * 4 + c


def _prep_wq(qw_slice):
    """[1024, N_loc] int32 -> [NJT, 128, N_loc] uint16 container tiles."""
    nloc = qw_slice.shape[1]
    qb = np.ascontiguousarray(qw_slice).view(np.uint8).reshape(1024, nloc, 4)
    # byte kp = 4*kk + b holds nibbles for k = 2kp (lo), 2kp+1 (hi)
    qb_kp = np.ascontiguousarray(qb.transpose(0, 2, 1)).reshape(4096, nloc)
    jt = np.arange(NJT)[:, None]
    p = np.arange(128)[None, :]
    kp0 = (p // 2) * 64 + (jt * 2 + (p % 2)) * 2      # [NJT, 128]
    b2 = np.stack([qb_kp[kp0], qb_kp[kp0 + 1]], axis=-1)  # [NJT,128,nloc,2] u8
    return np.ascontiguousarray(b2).view(np.uint16)[..., 0]


def _prep_xt(x):
    """x [32, 8192] fp16 -> xt [128, NJT*4*32] fp16, tile (jt,c) at cols (jt*4+c)*32."""
    kmap = _k_index_map()                              # [NJT, 128, 4]
    xf = x.astype(np.float32)
    xt = np.empty((128, NJT * 4 * TOKENS), dtype=np.float16)
    for jt in range(NJT):
        for c in range(4):
            blk = xf[:, kmap[jt, :, c]].T * (2.0 ** (-4 * c))   # [128, 32]
            xt[:, (jt * 4 + c) * TOKENS:(jt * 4 + c + 1) * TOKENS] = \
                blk.astype(np.float16)
    return xt


def _prep_r65(x):
    """[-A.T ; ones] -> [65, 32] fp16, A[t,g] = sum_{k in g} x[t,k] (fp32)."""
    A = x.astype(np.float32).reshape(TOKENS, G, GROUP_SIZE).sum(axis=2)  # [32, 64]
    r = np.empty((65, TOKENS), dtype=np.float16)
    r[:G] = (-A.T).astype(np.float16)
    r[G] = 1.0
    return r


# ---------------------------------------------------------------- device program

# every GP_EVERY-th (jt,c) scale pass runs on GpSimd instead of the DVE
# (DVE is the bottleneck engine; GpSimd TT is ~3.7x slower but otherwise idle)
GP_EVERY = 0


def _build_program(nloc, loop_r=1):
    import concourse.bacc as bacc
    import concourse.mybir as mybir
    import concourse.tile as tile
    from concourse.alu_op_type import AluOpType

    dt = mybir.dt
    nch = nloc // 128
    nzq = nloc // 4

    nc = bacc.Bacc("TRN2", target_bir_lowering=False, debug=False,
                   num_devices=N_CORES)

    wq_d = nc.dram_tensor("wq", [NJT, 128, nloc], dt.uint16, kind="ExternalInput")
    sexp_d = nc.dram_tensor("sexp", [128, nloc], dt.float16, kind="ExternalInput")
    sc_d = nc.dram_tensor("sc", [G, nloc], dt.float16, kind="ExternalInput")
    zq_d = nc.dram_tensor("zq", [G, nzq], dt.uint16, kind="ExternalInput")
    bias_d = nc.dram_tensor("bias", [1, nloc], dt.float16, kind="ExternalInput")
    xt_d = nc.dram_tensor("xt", [128, NJT * 4 * TOKENS], dt.float16,
                          kind="ExternalInput")
    r65_d = nc.dram_tensor("r65", [G + 1, TOKENS], dt.float16, kind="ExternalInput")
    out_d = nc.dram_tensor("out", [128, nch * TOKENS], dt.float16,
                           kind="ExternalOutput")

    with tile.TileContext(nc) as tc:
        with (
            tc.tile_pool(name="const", bufs=1) as const_pool,
            tc.tile_pool(name="wq", bufs=3) as wq_pool,
            tc.tile_pool(name="ext", bufs=4) as ext_pool,
            tc.tile_pool(name="sw", bufs=4) as sw_pool,
            tc.tile_pool(name="psum", bufs=1, space="PSUM") as psum_pool,
        ):
            def emit_body():
                sexp = const_pool.tile([128, nloc], dt.float16, tag="sexp")
                nc.sync.dma_start(sexp[:], sexp_d[:])
                sc = const_pool.tile([G, nloc], dt.float16, tag="sc")
                nc.sync.dma_start(sc[:], sc_d[:])
                zq = const_pool.tile([G, nzq], dt.uint16, tag="zq")
                nc.sync.dma_start(zq[:], zq_d[:])
                xt = const_pool.tile([128, NJT * 4 * TOKENS], dt.float16,
                                     tag="xt")
                nc.sync.dma_start(xt[:], xt_d[:])
                r65 = const_pool.tile([G + 1, TOKENS], dt.float16, tag="r65")
                nc.sync.dma_start(r65[:], r65_d[:])
                z65 = const_pool.tile([G + 1, nloc], dt.float16, tag="z65")
                nc.sync.dma_start(z65[G:G + 1, :], bias_d[:])

                # z1s rows: z65[g, n] = (z[g,n] + 1) * s[g,n], n = 4*nu + c
                # (bitwise DVE ops cannot cast dtypes on trn2 hw: extract
                # to uint16, arithmetic ops downstream do int->float)
                for c in range(4):
                    zt = const_pool.tile([G, nzq], dt.uint16, tag="ztmp")
                    nc.vector.tensor_scalar(
                        zt[:], zq[:], 4 * c, 15,
                        AluOpType.logical_shift_right, AluOpType.bitwise_and)
                    z65_v = z65[0:G, :].rearrange(
                        "g (n f) -> g f n", f=4)[:, c, :]
                    sc_v = sc[:].rearrange("g (n f) -> g f n", f=4)[:, c, :]
                    nc.vector.scalar_tensor_tensor(
                        z65_v, zt[:], 1.0, sc_v,
                        AluOpType.add, AluOpType.mult)

                psum = psum_pool.tile([128, nch * TOKENS], dt.float32,
                                      tag="acc")

                for jt in range(NJT):
                    wq_t = wq_pool.tile([128, nloc], dt.uint16)
                    nc.sync.dma_start(wq_t[:], wq_d[jt, :, :])
                    for c in range(4):
                        ext = ext_pool.tile([128, nloc], dt.uint16)
                        nc.vector.tensor_scalar(
                            ext[:], wq_t[:], MASKS[c], None,
                            AluOpType.bitwise_and)
                        sw = sw_pool.tile([128, nloc], dt.float16)
                        eng = (nc.gpsimd if GP_EVERY and
                               (jt * 4 + c) % GP_EVERY == GP_EVERY - 1
                               else nc.vector)
                        eng.tensor_tensor(
                            sw[:], ext[:], sexp[:], AluOpType.mult)
                        xcol = (jt * 4 + c) * TOKENS
                        for ci in range(nch):
                            # start=True clears has_written for the WHOLE
                            # 2KiB bank (16 regions of 32 fp32): issue
                            # exactly one start per bank; other regions'
                            # first matmuls overwrite-where-clear.
                            nc.tensor.matmul(
                                psum[:, ci * TOKENS:(ci + 1) * TOKENS],
                                sw[:, ci * 128:(ci + 1) * 128],
                                xt[:, xcol:xcol + TOKENS],
                                start=(jt == 0 and c == 0 and ci % 16 == 0),
                                stop=False)

                for ci in range(nch):
                    nc.tensor.matmul(
                        psum[:, ci * TOKENS:(ci + 1) * TOKENS],
                        z65[:, ci * 128:(ci + 1) * 128],
                        r65[:],
                        start=False,
                        stop=(ci == nch - 1 or ci % 16 == 15))

                stg = const_pool.tile([128, nch * TOKENS], dt.float16,
                                      tag="stg")
                nc.scalar.copy(stg[:], psum[:])
                nc.sync.dma_start(out_d[:], stg[:])

            if loop_r == 1:
                emit_body()
            else:
                with tc.For_i(0, loop_r, 1):
                    emit_body()

    nc.compile()
    return nc


def _get_program(nloc=N_LOC):
    if nloc not in _PROGRAM_CACHE:
        _PROGRAM_CACHE[nloc] = _build_program(nloc)
    return _PROGRAM_CACHE[nloc]


# ---------------------------------------------------------------- entry point

def make_in_maps(x, qweight, qzeros, scales, bias, nloc=N_LOC, n_cores=N_CORES):
    x = np.asarray(x)
    qweight = np.asarray(qweight)
    qzeros = np.asarray(qzeros)
    scales = np.asarray(scales)
    bias = np.asarray(bias)

    xt = _prep_xt(x)
    r65 = _prep_r65(x)
    in_maps = []
    for core in range(n_cores):
        n0, n1 = core * nloc, (core + 1) * nloc
        s_slice = np.ascontiguousarray(scales[:, n0:n1]).astype(np.float16)
        in_maps.append({
            "wq": _prep_wq(qweight[:, n0:n1]),
            "sexp": np.repeat(s_slice, 2, axis=0),
            "sc": s_slice,
            "zq": np.ascontiguousarray(
                qzeros[:, n0 // 8:n1 // 8]).view(np.uint16),
            "bias": np.ascontiguousarray(bias[n0:n1]).astype(
                np.float16).reshape(1, nloc),
            "xt": xt,
            "r65": r65,
        })
    return in_maps


def assemble_output(results, nloc=N_LOC, n_cores=N_CORES):
    nch = nloc // 128
    parts = []
    for core in range(n_cores):
        o = np.asarray(results[core]["out"])            # [128, nch*32] fp16
        o = o.reshape(128, nch, TOKENS).transpose(1, 0, 2).reshape(nloc, TOKENS)
        parts.append(o.T)                               # [32, nloc]
    return np.ascontiguousarray(np.concatenate(parts, axis=1))


def kernel(x, qweight, qzeros, scales, bias):
    from concourse.bass_utils import run_bass_kernel_spmd

    nc = _get_program()
    in_maps = make_in_maps(x, qweight, qzeros, scales, bias)
    res = run_bass_kernel_spmd(nc, in_maps, list(range(N_CORES)))
    return assemble_output(res.results)



# revision 11
# speedup vs baseline: 14.3378x; 10.1683x over previous
"""ExLlama q4 dequant + matmul, 8 trn2 cores — hybrid + pair-merged DVE ops.

Like kernel_d (hosted fp16 nibble planes for HJT jts + packed u16 containers
for the rest), but DVE tensor_tensor ops process TWO passes at once against a
resident doubled scale table [sexp|sexp], halving DVE op count; packed
extracts write into halves of a pair buffer.  For_i uses staggered_reset.
"""

import numpy as np

GROUP_SIZE = 128
IN_FEATURES = 8192
OUT_FEATURES = 28672
TOKENS = 32
N_CORES = 8
N_LOC = OUT_FEATURES // N_CORES          # 3584
NJT = IN_FEATURES // (GROUP_SIZE * 4)    # 16
G = IN_FEATURES // GROUP_SIZE            # 64
NPASS = NJT * 4
MMCH = 512

HJT = 8                                        # hosted jts (planes)
PJT = NJT - HJT
PACKED_JTS = sorted({round((i + 0.5) * NJT / PJT) % NJT
                     for i in range(PJT)}) if PJT else []
while len(PACKED_JTS) < PJT:
    PACKED_JTS.append(next(j for j in range(NJT) if j not in PACKED_JTS))
PACKED_JTS = sorted(PACKED_JTS[:PJT])
HOSTED_JTS = [j for j in range(NJT) if j not in PACKED_JTS]

PLANE_CH = 4                                   # hosted passes per plane DMA
PACK_CH = 2                                    # packed jts per container DMA
N_PLANE_DMA = max((HJT * 4) // PLANE_CH, 1)
N_PACK_DMA = (PJT + PACK_CH - 1) // PACK_CH if PJT else 1

# packed const layout (columns, fp16): [sexp2 | xt | z65 | r65]
C_SEXP = 0                                     # doubled: 2*N_LOC wide
C_XT = 2 * N_LOC
C_Z65 = C_XT + NPASS * TOKENS
C_R65 = C_Z65 + N_LOC
C_W = C_R65 + TOKENS

MASKS = (0x000F, 0x00F0, 0x0F00, 0xF000)
_PROGRAM_CACHE = {}


def _k_index_map():
    jt = np.arange(NJT)[:, None, None]
    p = np.arange(128)[None, :, None]
    c = np.arange(4)[None, None, :]
    return (p // 2) * GROUP_SIZE + (jt * 2 + (p % 2)) * 4 + c


def _containers(qw_slice):
    nloc = qw_slice.shape[1]
    qb = np.ascontiguousarray(qw_slice).view(np.uint8).reshape(1024, nloc, 4)
    qb_kp = np.ascontiguousarray(qb.transpose(0, 2, 1)).reshape(4096, nloc)
    jt = np.arange(NJT)[:, None]
    p = np.arange(128)[None, :]
    kp0 = (p // 2) * 64 + (jt * 2 + (p % 2)) * 2
    b2 = np.stack([qb_kp[kp0], qb_kp[kp0 + 1]], axis=-1)
    return np.ascontiguousarray(b2).view(np.uint16)[..., 0]


def _prep_weights(qw_slice):
    nloc = qw_slice.shape[1]
    wq = _containers(qw_slice)
    hp = np.empty((max(HJT * 4, PLANE_CH), 128, nloc), dtype=np.float16)
    for i, jt in enumerate(HOSTED_JTS):
        for c in range(4):
            hp[i * 4 + c] = ((wq[jt] >> (4 * c)) & 15).astype(np.float16)
    hp = hp[:N_PLANE_DMA * PLANE_CH]
    hp = hp.reshape(N_PLANE_DMA, PLANE_CH, 128, nloc).transpose(0, 2, 1, 3)
    planes = np.ascontiguousarray(hp).reshape(N_PLANE_DMA, 128,
                                              PLANE_CH * nloc)
    pk = np.zeros((N_PACK_DMA * PACK_CH, 128, nloc), dtype=np.uint16)
    for i, jt in enumerate(PACKED_JTS):
        pk[i] = wq[jt]
    pk = pk.reshape(N_PACK_DMA, PACK_CH, 128, nloc).transpose(0, 2, 1, 3)
    packs = np.ascontiguousarray(pk).reshape(N_PACK_DMA, 128, PACK_CH * nloc)
    return planes, packs


def _prep_const(x, qz_slice, s_slice, b_slice):
    nloc = s_slice.shape[1]
    cst = np.zeros((128, C_W), dtype=np.float16)
    sexp = np.repeat(s_slice.astype(np.float16), 2, axis=0)
    cst[:, C_SEXP:C_SEXP + nloc] = sexp
    cst[:, C_SEXP + nloc:C_SEXP + 2 * nloc] = sexp
    kmap = _k_index_map()
    xf = x.astype(np.float32)
    packed = set(PACKED_JTS)
    for jt in range(NJT):
        for c in range(4):
            col = C_XT + (jt * 4 + c) * TOKENS
            scale = (2.0 ** (-4 * c)) if jt in packed else 1.0
            cst[:, col:col + TOKENS] = (xf[:, kmap[jt, :, c]].T * scale
                                        ).astype(np.float16)
    shifts = (np.arange(8, dtype=np.uint32) * 4)[None, None, :]
    z = ((qz_slice.astype(np.uint32)[:, :, None] >> shifts) & 15)
    z = z.reshape(G, nloc).astype(np.float32)
    cst[:G, C_Z65:C_Z65 + nloc] = ((z + 1.0) * s_slice.astype(np.float32)
                                   ).astype(np.float16)
    cst[G, C_Z65:C_Z65 + nloc] = b_slice
    A = x.astype(np.float32).reshape(TOKENS, G, GROUP_SIZE).sum(axis=2)
    cst[:G, C_R65:C_R65 + TOKENS] = (-A.T).astype(np.float16)
    cst[G, C_R65:C_R65 + TOKENS] = 1.0
    return cst


def _build_program(nloc, loop_r=1):
    import concourse.bacc as bacc
    import concourse.mybir as mybir
    import concourse.tile as tile
    from concourse.alu_op_type import AluOpType

    dt = mybir.dt
    nch = nloc // MMCH

    nc = bacc.Bacc("TRN2", target_bir_lowering=False, debug=False,
                   num_devices=N_CORES)

    wp_d = nc.dram_tensor("wp", [N_PLANE_DMA, 128, PLANE_CH * nloc],
                          dt.float16, kind="ExternalInput")
    pk_d = nc.dram_tensor("pk", [N_PACK_DMA, 128, PACK_CH * nloc],
                          dt.uint16, kind="ExternalInput")
    cst_d = nc.dram_tensor("cst", [128, C_W], dt.float16,
                           kind="ExternalInput")
    out_d = nc.dram_tensor("out", [TOKENS, nloc], dt.float16,
                           kind="ExternalOutput")

    with tile.TileContext(nc) as tc:
        with (
            tc.tile_pool(name="const", bufs=1) as const_pool,
            tc.tile_pool(name="wp", bufs=2) as wp_pool,
            tc.tile_pool(name="pk", bufs=2) as pk_pool,
            tc.tile_pool(name="ext", bufs=2) as ext_pool,
            tc.tile_pool(name="sw", bufs=3) as sw_pool,
            tc.tile_pool(name="psum", bufs=1, space="PSUM") as psum_pool,
        ):
            def emit_body():
                cst = const_pool.tile([128, C_W], dt.float16, tag="cst")
                nc.sync.dma_start(cst[:, C_SEXP:C_SEXP + 2 * nloc],
                                  cst_d[:, C_SEXP:C_SEXP + 2 * nloc])
                nc.sync.dma_start(cst[:, C_XT:C_W], cst_d[:, C_XT:C_W])
                sexp2 = cst[:, C_SEXP:C_SEXP + 2 * nloc]
                psum = psum_pool.tile([TOKENS, nch * MMCH], dt.float32,
                                      tag="acc")

                state = {"first": True}

                def matmuls(ip, sw, off):
                    xcol = C_XT + ip * TOKENS
                    for ci in range(nch):
                        nc.tensor.matmul(
                            psum[:, ci * MMCH:(ci + 1) * MMCH],
                            cst[:, xcol:xcol + TOKENS],
                            sw[:, off + ci * MMCH:off + (ci + 1) * MMCH],
                            start=state["first"],
                            stop=False)
                    state["first"] = False

                hosted_seq = [(jt, c) for jt in HOSTED_JTS for c in range(4)]
                hp_i = pk_i = 0
                plane_tiles = []
                pack_tiles = []
                hpos = 0
                ppos = 0
                total = len(hosted_seq) + 4 * len(PACKED_JTS)
                emitted = 0
                while emitted < total:
                    want_hosted = (hpos * total <=
                                   (hpos + 4 * ppos) * max(len(hosted_seq), 1))
                    if want_hosted and hpos < len(hosted_seq):
                        if hpos % PLANE_CH == 0:
                            t = wp_pool.tile([128, PLANE_CH * nloc],
                                             dt.float16)
                            nc.sync.dma_start(t[:], wp_d[hp_i, :, :])
                            plane_tiles.append(t)
                            hp_i += 1
                        t = plane_tiles[-1]
                        j = hpos % PLANE_CH          # 0 or 2 (pair-aligned)
                        sw = sw_pool.tile([128, 2 * nloc], dt.float16)
                        nc.vector.tensor_tensor(
                            sw[:], t[:, j * nloc:(j + 2) * nloc],
                            sexp2, AluOpType.mult)
                        for u in range(2):
                            jt, c = hosted_seq[hpos + u]
                            matmuls(jt * 4 + c, sw, u * nloc)
                        hpos += 2
                        emitted += 2
                    elif ppos < len(PACKED_JTS):
                        if ppos % PACK_CH == 0:
                            t = pk_pool.tile([128, PACK_CH * nloc], dt.uint16)
                            nc.sync.dma_start(t[:], pk_d[pk_i, :, :])
                            pack_tiles.append(t)
                            pk_i += 1
                        t = pack_tiles[-1]
                        j = ppos % PACK_CH
                        jt = PACKED_JTS[ppos]
                        wsl = t[:, j * nloc:(j + 1) * nloc]
                        for cp in range(2):          # c pairs (0,1), (2,3)
                            ext = ext_pool.tile([128, 2 * nloc], dt.uint16)
                            nc.vector.tensor_scalar(
                                ext[:, 0:nloc], wsl, MASKS[2 * cp], None,
                                AluOpType.bitwise_and)
                            nc.vector.tensor_scalar(
                                ext[:, nloc:2 * nloc], wsl,
                                MASKS[2 * cp + 1], None,
                                AluOpType.bitwise_and)
                            sw = sw_pool.tile([128, 2 * nloc], dt.float16)
                            nc.vector.tensor_tensor(
                                sw[:], ext[:], sexp2, AluOpType.mult)
                            for u in range(2):
                                matmuls(jt * 4 + 2 * cp + u, sw, u * nloc)
                            emitted += 2
                        ppos += 1
                    else:
                        continue

                for ci in range(nch):
                    nc.tensor.matmul(
                        psum[:, ci * MMCH:(ci + 1) * MMCH],
                        cst[0:G + 1, C_R65:C_R65 + TOKENS],
                        cst[0:G + 1, C_Z65 + ci * MMCH:C_Z65 + (ci + 1) * MMCH],
                        start=False,
                        stop=True)

                stg = const_pool.tile([TOKENS, nch * MMCH], dt.float16,
                                      tag="stg")
                nc.scalar.copy(stg[:], psum[:])
                nc.sync.dma_start(out_d[:], stg[:])

            if loop_r == 1:
                emit_body()
            else:
                with tc.For_i(0, loop_r, 1, staggered_reset=True):
                    emit_body()

    nc.compile()
    return nc


def _get_program(nloc=N_LOC):
    if nloc not in _PROGRAM_CACHE:
        _PROGRAM_CACHE[nloc] = _build_program(nloc)
    return _PROGRAM_CACHE[nloc]


def make_in_maps(x, qweight, qzeros, scales, bias, nloc=N_LOC, n_cores=N_CORES):
    x = np.asarray(x)
    qweight = np.asarray(qweight)
    qzeros = np.asarray(qzeros)
    scales = np.asarray(scales)
    bias = np.asarray(bias)

    in_maps = []
    for core in range(n_cores):
        n0, n1 = core * nloc, (core + 1) * nloc
        s_slice = np.ascontiguousarray(scales[:, n0:n1]).astype(np.float16)
        qz_slice = np.ascontiguousarray(qzeros[:, n0 // 8:n1 // 8]).view(
            np.uint32)
        b_slice = np.ascontiguousarray(bias[n0:n1]).astype(np.float16)
        planes, packs = _prep_weights(qweight[:, n0:n1])
        in_maps.append({
            "wp": planes,
            "pk": packs,
            "cst": _prep_const(x, qz_slice, s_slice, b_slice),
        })
    return in_maps


def assemble_output(results, nloc=N_LOC, n_cores=N_CORES):
    parts = [np.asarray(results[core]["out"]) for core in range(n_cores)]
    return np.ascontiguousarray(np.concatenate(parts, axis=1))


def kernel(x, qweight, qzeros, scales, bias):
    from concourse.bass_utils import run_bass_kernel_spmd

    nc = _get_program()
    in_maps = make_in_maps(x, qweight, qzeros, scales, bias)
    res = run_bass_kernel_spmd(nc, in_maps, list(range(N_CORES)))
    return assemble_output(res.results)
